# revision 10
# baseline (speedup 1.0000x reference)
"""Multi-head attention (B=4, S=1024, E=512, A=64, H=8) on 8 Trainium2 cores.

Sharding: core c -> (batch b = c//2, head-group g = c%2 covering heads
4g..4g+3). Each core computes its 4 heads end-to-end plus a *partial*
output projection (out = sum_h heads_h @ wo_h decomposes over heads), so
the host just sums core pairs and adds bo. No collectives.

Device layout (per core) keeps everything transposed so no on-device
transposes are needed:
  xt   [E=512, S=1024]  (host passes x^T)
  qT,kT [A=64, S] per head, pair-stacked into [128, 2, 1024] tiles
  v    [S, A] natural, with a ones column appended -> AV matmul row 64
       accumulates the softmax denominator for free
  scores^T tiles [t_chunk=128, s_block<=512]: causal masking skips
       upper-triangle tiles and shortens diagonal ones; only a 128x128
       sub-block per diagonal tile needs an elementwise mask multiply.
"""

import os
import numpy as np

B, S, E, A, H = 4, 1024, 512, 64, 8
HL = 4  # heads per core
N_CORES = 8
P = 128

_compiled = None


def _f32(x):
    return np.ascontiguousarray(x, dtype=np.float32)


def _build_masks():
    # dmask[d, tp, sf] = 1.0 if (128*d + tp) <= sf else 0 ; d = t_chunk - 4*s_block
    d = np.arange(4)[:, None, None]
    tp = np.arange(P)[None, :, None]
    sf = np.arange(512)[None, None, :]
    return ((P * d + tp) <= sf).astype(np.float32)


def _shard_inputs(x, wq, bq, wk, bk, wv, bv, wo):
    """Build the 8 per-core input dicts."""
    dmask = _build_masks()
    in_maps = []
    for c in range(N_CORES):
        b, g = divmod(c, 2)
        hs = slice(4 * g, 4 * g + 4)
        wq_l, wk_l, wv_l = wq[hs], wk[hs], wv[hs]          # [4, E, A]
        m = {
            "xt": _f32(x[b].T),                             # [E, S]
            "wq_s": _f32(np.transpose(wq_l, (1, 0, 2)).reshape(E, HL * A)),
            "wk_s": _f32(np.transpose(wk_l, (1, 0, 2)).reshape(E, HL * A)),
            "wv_s": _f32(np.transpose(wv_l, (1, 0, 2)).reshape(E, HL * A)),
            "bv_s": _f32(bv[hs].reshape(1, HL * A)),
            "bq_s": _f32(bq[hs].reshape(2, 2 * A).T),       # [128, 2] pair-stacked
            "bk_s": _f32(bk[hs].reshape(2, 2 * A).T),
            "wo_s": _f32(wo[g * HL * A:(g + 1) * HL * A]
                         .reshape(HL, A, A).transpose(1, 0, 2)),  # [64, 4, 64]
            "dmask": dmask,
        }
        in_maps.append(m)
    return in_maps


def _emit(tc, aps):
    from contextlib import ExitStack
    import concourse.bass as bass
    import concourse.mybir as mybir
    from concourse.bass import ts

    nc = tc.nc
    f32 = mybir.dt.float32
    Identity = mybir.ActivationFunctionType.Identity
    Exp = mybir.ActivationFunctionType.Exp

    xt, wq_s, wk_s, wv_s, bv_s, bq_s, bk_s, wo_s, dmask, out = (
        aps["xt"], aps["wq_s"], aps["wk_s"], aps["wv_s"], aps["bv_s"],
        aps["bq_s"], aps["bk_s"], aps["wo_s"], aps["dmask"], aps["out"],
    )

    ctx = ExitStack()
    const = ctx.enter_context(tc.tile_pool(name="const", bufs=1))
    ppool = ctx.enter_context(tc.tile_pool(name="p", bufs=10))
    rpool = ctx.enter_context(tc.tile_pool(name="r", bufs=2))
    dpool = ctx.enter_context(tc.tile_pool(name="dram", bufs=2, space="DRAM"))
    mm_ps = ctx.enter_context(tc.tile_pool(name="mmps", bufs=2, space="PSUM"))
    sc_ps = ctx.enter_context(tc.tile_pool(name="scps", bufs=3, space="PSUM"))
    av_ps = ctx.enter_context(tc.tile_pool(name="avps", bufs=1, space="PSUM"))
    o_ps = ctx.enter_context(tc.tile_pool(name="ops", bufs=1, space="PSUM"))

    # ---- constant loads ----
    xt_sb = const.tile([P, 4, S], f32, tag="xt")
    nc.sync.dma_start(xt_sb[:], xt.rearrange("(k p) s -> p k s", p=P))
    wq_sb = const.tile([P, 4, 256], f32, tag="wq")
    nc.sync.dma_start(wq_sb[:], wq_s.rearrange("(k p) n -> p k n", p=P))
    wk_sb = const.tile([P, 4, 256], f32, tag="wk")
    nc.sync.dma_start(wk_sb[:], wk_s.rearrange("(k p) n -> p k n", p=P))
    wv_sb = const.tile([P, 4, 256], f32, tag="wv")
    nc.sync.dma_start(wv_sb[:], wv_s.rearrange("(k p) n -> p k n", p=P))
    bv_sb = const.tile([1, 256], f32, tag="bv")
    nc.sync.dma_start(bv_sb[:], bv_s[:])
    bq_sb = const.tile([P, 2], f32, tag="bq")
    nc.sync.dma_start(bq_sb[:], bq_s[:])
    bk_sb = const.tile([P, 2], f32, tag="bk")
    nc.sync.dma_start(bk_sb[:], bk_s[:])
    wo_sb = const.tile([64, 4, 64], f32, tag="wo")
    nc.sync.dma_start(wo_sb[:], wo_s[:])
    mk_sb = const.tile([P, 4, 512], f32, tag="mk")
    nc.sync.dma_start(mk_sb[:], dmask.rearrange("d p s -> p d s"))

    ones_sb = const.tile([1, S], f32, tag="ones")
    nc.vector.memset(ones_sb[:], 1.0)

    qt_sb = const.tile([P, 2, S], f32, tag="qt")   # pair-stacked q^T
    kt_sb = const.tile([P, 2, S], f32, tag="kt")
    vv_sb = const.tile([P, HL, 8, A + 1], f32, tag="vv")  # v|ones per head/t-chunk
    nc.vector.memset(vv_sb[:, :, :, A:A + 1], 1.0)
    hn_sb = [const.tile([A, S], f32, tag=f"hn{h}", name=f"hn{h}") for h in range(HL)]
    osb = const.tile([P, 8, A], f32, tag="osb")

    # ---- V projection: v[s, a] for all 4 heads at once (N=256) ----
    for t in range(8):
        pv = mm_ps.tile([P, 512], f32, tag="mm")
        for k in range(4):
            nc.tensor.matmul(pv[:, :256], lhsT=xt_sb[:, k, ts(t, P)],
                             rhs=wv_sb[:, k, :], start=(k == 0), stop=False)
        nc.tensor.matmul(pv[:, :256], lhsT=ones_sb[:, ts(t, P)],
                         rhs=bv_sb[:], start=False, stop=True)
        for h in range(HL):
            nc.vector.tensor_copy(vv_sb[:, h, t, 0:A], pv[:, A * h:A * h + A])

    # ---- Q/K projections: q^T, k^T pair-stacked [128, S] (2 heads x 64) ----
    for wsb, bsb, dst in ((wq_sb, bq_sb, qt_sb), (wk_sb, bk_sb, kt_sb)):
        for p in range(2):
            for blk in range(2):
                pq = mm_ps.tile([P, 512], f32, tag="mm")
                for k in range(4):
                    nc.tensor.matmul(pq[0:64, :], lhsT=wsb[:, k, 128 * p:128 * p + 64],
                                     rhs=xt_sb[:, k, ts(blk, 512)],
                                     start=(k == 0), stop=(k == 3),
                                     skip_group_check=True)
                    nc.tensor.matmul(pq[64:128, :], lhsT=wsb[:, k, 128 * p + 64:128 * p + 128],
                                     rhs=xt_sb[:, k, ts(blk, 512)],
                                     start=(k == 0), stop=(k == 3),
                                     skip_group_check=True)
                nc.scalar.activation(dst[:, p, ts(blk, 512)], pq[:, :], Identity,
                                     bias=bsb[:, p:p + 1], scale=1.0)

    # ---- attention per head pair; scores^T layout [t, s] ----
    for p in range(2):
        for j in range(2):  # s_block
            n_i = 4 * j + 4
            pts = {}
            for i in range(n_i):  # t_chunk
                d = i - 4 * j
                col0 = 128 * d if d > 0 else 0
                for loc in range(2):
                    pr = 64 * loc
                    sc = sc_ps.tile([P, 512], f32, tag="sc")
                    nc.tensor.matmul(sc[:, col0:], lhsT=kt_sb[pr:pr + 64, p, ts(i, P)],
                                     rhs=qt_sb[pr:pr + 64, p, 512 * j + col0:512 * (j + 1)],
                                     start=True, stop=True)
                    pt = ppool.tile([P, 512], f32, tag="p")
                    nc.scalar.activation(pt[:, col0:], sc[:, col0:], Exp, scale=0.125)
                    if d >= 0:
                        nc.vector.tensor_mul(pt[:, col0:col0 + 128], pt[:, col0:col0 + 128],
                                             mk_sb[:, d, col0:col0 + 128])
                    pts[(i, loc)] = pt
            for loc in range(2):
                h = 2 * p + loc
                av = av_ps.tile([A + 1, 512], f32, tag="av")
                for i in range(n_i):
                    d = i - 4 * j
                    col0 = 128 * d if d > 0 else 0
                    nc.tensor.matmul(av[:, col0:], lhsT=vv_sb[:, h, i, :],
                                     rhs=pts[(i, loc)][:, col0:],
                                     start=(i == 0), stop=(i == n_i - 1))
                rt = rpool.tile([P, 512], f32, tag="r")
                nc.vector.reciprocal(rt[64:65, :], av[64:65, :])
                # partition-broadcast the reciprocal row via a DRAM bounce
                # (SBUF DMA sources reject stride-0 partition dims; DRAM ones don't)
                dscr = dpool.tile([1, 512], f32, tag="dscr")
                nc.sync.dma_start(dscr[:], rt[64:65, :])
                nc.sync.dma_start(rt[0:64, :], dscr[0:1, :].to_broadcast((64, 512)))
                nc.vector.tensor_mul(hn_sb[h][:, ts(j, 512)], av[0:A, :], rt[0:64, :])

    # ---- partial output projection: out[s, :] = sum_h heads_h @ wo_h ----
    for t in range(8):
        po = o_ps.tile([P, A], f32, tag="o")
        for h in range(HL):
            nc.tensor.matmul(po[:], lhsT=hn_sb[h][:, ts(t, P)], rhs=wo_sb[:, h, :],
                             start=(h == 0), stop=(h == HL - 1))
        nc.vector.tensor_copy(osb[:, t, :], po[:])
    nc.sync.dma_start(out.rearrange("(c p) n -> p c n", p=P), osb[:])
    ctx.close()


def _build():
    global _compiled
    if _compiled is not None:
        return _compiled
    import concourse.bacc as bacc
    import concourse.mybir as mybir
    import concourse.tile as tile

    nc = bacc.Bacc("TRN2", target_bir_lowering=False, debug=False,
                   num_devices=N_CORES)
    f32 = mybir.dt.float32
    aps = {
        "xt": nc.dram_tensor("xt", [E, S], f32, kind="ExternalInput").ap(),
        "wq_s": nc.dram_tensor("wq_s", [E, HL * A], f32, kind="ExternalInput").ap(),
        "wk_s": nc.dram_tensor("wk_s", [E, HL * A], f32, kind="ExternalInput").ap(),
        "wv_s": nc.dram_tensor("wv_s", [E, HL * A], f32, kind="ExternalInput").ap(),
        "bv_s": nc.dram_tensor("bv_s", [1, HL * A], f32, kind="ExternalInput").ap(),
        "bq_s": nc.dram_tensor("bq_s", [P, 2], f32, kind="ExternalInput").ap(),
        "bk_s": nc.dram_tensor("bk_s", [P, 2], f32, kind="ExternalInput").ap(),
        "wo_s": nc.dram_tensor("wo_s", [A, HL, A], f32, kind="ExternalInput").ap(),
        "dmask": nc.dram_tensor("dmask", [4, P, 512], f32, kind="ExternalInput").ap(),
        "out": nc.dram_tensor("out", [S, A], f32, kind="ExternalOutput").ap(),
    }
    with tile.TileContext(nc) as tc:
        _emit(tc, aps)
    nc.compile()
    _compiled = nc
    return nc


def _install_ntff_shim():
    """run_bass_kernel_spmd's trace path imports antenv.axon_hooks, which
    this container lacks; trn_agent_boot ships the ctypes equivalent."""
    import sys
    import types
    try:
        import antenv.axon_hooks  # noqa: F401
        return
    except ImportError:
        pass
    try:
        from trn_agent_boot.trn_boot import _ntff_profile_via_ctypes
        hook = _ntff_profile_via_ctypes('/opt/axon/libaxon_pjrt.so')
    except Exception:
        hook = None
    mod = types.ModuleType('antenv.axon_hooks')
    mod.get_axon_ntff_profile_hook = lambda: hook
    mod.set_axon_ntff_profile_hook = lambda h: None
    pkg = types.ModuleType('antenv')
    pkg.axon_hooks = mod
    sys.modules.setdefault('antenv', pkg)
    sys.modules['antenv.axon_hooks'] = mod


def kernel(x, mask, wq, bq, wk, bk, wv, bv, wo, bo, _trace=False):
    x, wq, bq, wk, bk, wv, bv, wo, bo = (
        np.asarray(a) for a in (x, wq, bq, wk, bk, wv, bv, wo, bo))
    from concourse.bass_utils import run_bass_kernel_spmd

    nc = _build()
    in_maps = _shard_inputs(x, wq, bq, wk, bk, wv, bv, wo)
    kw = {}
    if _trace:
        _install_ntff_shim()
        kw = dict(trace=True, trace_cores=list(range(N_CORES)))
    res = run_bass_kernel_spmd(nc, in_maps, list(range(N_CORES)), **kw)
    parts = [res.results[c]["out"] for c in range(N_CORES)]
    out = np.stack([parts[2 * b] + parts[2 * b + 1] for b in range(B)])
    out = out + np.asarray(bo, dtype=np.float32)[None, None, :]
    if _trace:
        kernel.last_exec_time_ns = res.exec_time_ns
        kernel.last_mean_exec_time_ns = res.mean_exec_time_ns
    return out.astype(np.float32)


# revision 11
# speedup vs baseline: 1.0021x; 1.0021x over previous
"""Multi-head attention (B=4, S=1024, E=512, A=64, H=8) on 8 Trainium2 cores.

Sharding: core c -> (batch b = c//2, head-group g = c%2 covering heads
4g..4g+3). Each core computes its 4 heads end-to-end plus a *partial*
output projection (out = sum_h heads_h @ wo_h decomposes over heads), so
the host just sums core pairs and adds bo. No collectives.

Device layout (per core) keeps everything transposed so no on-device
transposes are needed:
  xt   [E=512, S=1024]  (host passes x^T)
  qT,kT [A=64, S] per head, pair-stacked into [128, 2, 1024] tiles
  v    [S, A] natural, with a ones column appended -> AV matmul row 64
       accumulates the softmax denominator for free
  scores^T tiles [t_chunk=128, s_block<=512]: causal masking skips
       upper-triangle tiles and shortens diagonal ones; only a 128x128
       sub-block per diagonal tile needs an elementwise mask multiply.
"""

import os
import numpy as np

B, S, E, A, H = 4, 1024, 512, 64, 8
HL = 4  # heads per core
N_CORES = 8
P = 128

_compiled = None


def _f32(x):
    return np.ascontiguousarray(x, dtype=np.float32)


def _build_masks():
    # dmask[d, tp, sf] = 1.0 if (128*d + tp) <= sf else 0 ; d = t_chunk - 4*s_block
    d = np.arange(4)[:, None, None]
    tp = np.arange(P)[None, :, None]
    sf = np.arange(512)[None, None, :]
    return ((P * d + tp) <= sf).astype(np.float32)


def _shard_inputs(x, wq, bq, wk, bk, wv, bv, wo):
    """Build the 8 per-core input dicts."""
    dmask = _build_masks()
    in_maps = []
    for c in range(N_CORES):
        b, g = divmod(c, 2)
        hs = slice(4 * g, 4 * g + 4)
        wq_l, wk_l, wv_l = wq[hs], wk[hs], wv[hs]          # [4, E, A]
        m = {
            "xt": _f32(x[b].T),                             # [E, S]
            "wq_s": _f32(np.transpose(wq_l, (1, 0, 2)).reshape(E, HL * A)),
            "wk_s": _f32(np.transpose(wk_l, (1, 0, 2)).reshape(E, HL * A)),
            "wv_s": _f32(np.transpose(wv_l, (1, 0, 2)).reshape(E, HL * A)),
            "bv_s": _f32(bv[hs].reshape(1, HL * A)),
            "bq_s": _f32(bq[hs].reshape(2, 2 * A).T),       # [128, 2] pair-stacked
            "bk_s": _f32(bk[hs].reshape(2, 2 * A).T),
            "wo_s": _f32(wo[g * HL * A:(g + 1) * HL * A]
                         .reshape(HL, A, A).transpose(1, 0, 2)),  # [64, 4, 64]
            "dmask": dmask,
        }
        in_maps.append(m)
    return in_maps


def _emit(tc, aps):
    from contextlib import ExitStack
    import concourse.bass as bass
    import concourse.mybir as mybir
    from concourse.bass import ts

    nc = tc.nc
    f32 = mybir.dt.float32
    Identity = mybir.ActivationFunctionType.Identity
    Exp = mybir.ActivationFunctionType.Exp

    xt, wq_s, wk_s, wv_s, bv_s, bq_s, bk_s, wo_s, dmask, out = (
        aps["xt"], aps["wq_s"], aps["wk_s"], aps["wv_s"], aps["bv_s"],
        aps["bq_s"], aps["bk_s"], aps["wo_s"], aps["dmask"], aps["out"],
    )

    ctx = ExitStack()
    const = ctx.enter_context(tc.tile_pool(name="const", bufs=1))
    ppool = ctx.enter_context(tc.tile_pool(name="p", bufs=10))
    rpool = ctx.enter_context(tc.tile_pool(name="r", bufs=2))
    dpool = ctx.enter_context(tc.tile_pool(name="dram", bufs=2, space="DRAM"))
    mm_ps = ctx.enter_context(tc.tile_pool(name="mmps", bufs=2, space="PSUM"))
    sc_ps = ctx.enter_context(tc.tile_pool(name="scps", bufs=3, space="PSUM"))
    av_ps = ctx.enter_context(tc.tile_pool(name="avps", bufs=1, space="PSUM"))
    o_ps = ctx.enter_context(tc.tile_pool(name="ops", bufs=1, space="PSUM"))

    # ---- constant loads ----
    xt_sb = const.tile([P, 4, S], f32, tag="xt")
    nc.sync.dma_start(xt_sb[:], xt.rearrange("(k p) s -> p k s", p=P))
    wq_sb = const.tile([P, 4, 256], f32, tag="wq")
    nc.sync.dma_start(wq_sb[:], wq_s.rearrange("(k p) n -> p k n", p=P))
    wk_sb = const.tile([P, 4, 256], f32, tag="wk")
    nc.sync.dma_start(wk_sb[:], wk_s.rearrange("(k p) n -> p k n", p=P))
    wv_sb = const.tile([P, 4, 256], f32, tag="wv")
    nc.sync.dma_start(wv_sb[:], wv_s.rearrange("(k p) n -> p k n", p=P))
    bv_sb = const.tile([1, 256], f32, tag="bv")
    nc.sync.dma_start(bv_sb[:], bv_s[:])
    bq_sb = const.tile([P, 2], f32, tag="bq")
    nc.sync.dma_start(bq_sb[:], bq_s[:])
    bk_sb = const.tile([P, 2], f32, tag="bk")
    nc.sync.dma_start(bk_sb[:], bk_s[:])
    wo_sb = const.tile([64, 4, 64], f32, tag="wo")
    nc.sync.dma_start(wo_sb[:], wo_s[:])
    mk_sb = const.tile([P, 4, 512], f32, tag="mk")
    nc.sync.dma_start(mk_sb[:], dmask.rearrange("d p s -> p d s"))

    ones_sb = const.tile([1, S], f32, tag="ones")
    nc.vector.memset(ones_sb[:], 1.0)

    qt_sb = const.tile([P, 2, S], f32, tag="qt")   # pair-stacked q^T
    kt_sb = const.tile([P, 2, S], f32, tag="kt")
    vv_sb = const.tile([P, HL, 8, A + 1], f32, tag="vv")  # v|ones per head/t-chunk
    nc.vector.memset(vv_sb[:, :, :, A:A + 1], 1.0)
    hn_sb = [const.tile([A, S], f32, tag=f"hn{h}", name=f"hn{h}") for h in range(HL)]
    osb = const.tile([P, 8, A], f32, tag="osb")

    # ---- V projection: v[s, a] for all 4 heads at once (N=256) ----
    for t in range(8):
        pv = mm_ps.tile([P, 512], f32, tag="mm")
        for k in range(4):
            nc.tensor.matmul(pv[:, :256], lhsT=xt_sb[:, k, ts(t, P)],
                             rhs=wv_sb[:, k, :], start=(k == 0), stop=False)
        nc.tensor.matmul(pv[:, :256], lhsT=ones_sb[:, ts(t, P)],
                         rhs=bv_sb[:], start=False, stop=True)
        for h in range(HL):
            nc.vector.tensor_copy(vv_sb[:, h, t, 0:A], pv[:, A * h:A * h + A])

    # ---- Q/K projections: q^T, k^T pair-stacked [128, S] (2 heads x 64) ----
    for wsb, bsb, dst in ((wq_sb, bq_sb, qt_sb), (wk_sb, bk_sb, kt_sb)):
        for p in range(2):
            for blk in range(2):
                pq = mm_ps.tile([P, 512], f32, tag="mm")
                for k in range(4):
                    nc.tensor.matmul(pq[0:64, :], lhsT=wsb[:, k, 128 * p:128 * p + 64],
                                     rhs=xt_sb[:, k, ts(blk, 512)],
                                     start=(k == 0), stop=(k == 3),
                                     skip_group_check=True)
                    nc.tensor.matmul(pq[64:128, :], lhsT=wsb[:, k, 128 * p + 64:128 * p + 128],
                                     rhs=xt_sb[:, k, ts(blk, 512)],
                                     start=(k == 0), stop=(k == 3),
                                     skip_group_check=True)
                nc.scalar.activation(dst[:, p, ts(blk, 512)], pq[:, :], Identity,
                                     bias=bsb[:, p:p + 1], scale=1.0)

    # ---- attention per head pair; scores^T layout [t, s] ----
    for p in range(2):
        for j in range(2):  # s_block
            n_i = 4 * j + 4
            pts = {}
            for i in range(n_i):  # t_chunk
                d = i - 4 * j
                col0 = 128 * d if d > 0 else 0
                for loc in range(2):
                    pr = 64 * loc
                    sc = sc_ps.tile([P, 512], f32, tag="sc")
                    nc.tensor.matmul(sc[:, col0:], lhsT=kt_sb[pr:pr + 64, p, ts(i, P)],
                                     rhs=qt_sb[pr:pr + 64, p, 512 * j + col0:512 * (j + 1)],
                                     start=True, stop=True)
                    pt = ppool.tile([P, 512], f32, tag="p")
                    nc.scalar.activation(pt[:, col0:], sc[:, col0:], Exp, scale=0.125)
                    if d >= 0:
                        nc.vector.tensor_mul(pt[:, col0:col0 + 128], pt[:, col0:col0 + 128],
                                             mk_sb[:, d, col0:col0 + 128])
                    pts[(i, loc)] = pt
            for loc in range(2):
                h = 2 * p + loc
                av = av_ps.tile([A + 1, 512], f32, tag="av")
                for i in range(n_i):
                    d = i - 4 * j
                    col0 = 128 * d if d > 0 else 0
                    nc.tensor.matmul(av[:, col0:], lhsT=vv_sb[:, h, i, :],
                                     rhs=pts[(i, loc)][:, col0:],
                                     start=(i == 0), stop=(i == n_i - 1))
                rt = rpool.tile([P, 512], f32, tag="r")
                nc.vector.reciprocal(rt[64:65, :], av[64:65, :])
                # partition-broadcast the reciprocal row via a DRAM bounce
                # (SBUF DMA sources reject stride-0 partition dims; DRAM ones don't)
                dscr = dpool.tile([1, 512], f32, tag="dscr")
                nc.sync.dma_start(dscr[:], rt[64:65, :])
                nc.sync.dma_start(rt[0:64, :], dscr[0:1, :].to_broadcast((64, 512)))
                nc.vector.tensor_mul(hn_sb[h][:, ts(j, 512)], av[0:A, :], rt[0:64, :])

    # ---- partial output projection: out[s, :] = sum_h heads_h @ wo_h ----
    for t in range(8):
        po = o_ps.tile([P, A], f32, tag="o")
        for h in range(HL):
            nc.tensor.matmul(po[:], lhsT=hn_sb[h][:, ts(t, P)], rhs=wo_sb[:, h, :],
                             start=(h == 0), stop=(h == HL - 1))
        nc.vector.tensor_copy(osb[:, t, :], po[:])
    nc.sync.dma_start(out.rearrange("(c p) n -> p c n", p=P), osb[:])
    ctx.close()


def _build():
    global _compiled
    if _compiled is not None:
        return _compiled
    import concourse.bacc as bacc
    import concourse.mybir as mybir
    import concourse.tile as tile

    nc = bacc.Bacc("TRN2", target_bir_lowering=False, debug=False,
                   num_devices=N_CORES)
    f32 = mybir.dt.float32
    aps = {
        "xt": nc.dram_tensor("xt", [E, S], f32, kind="ExternalInput").ap(),
        "wq_s": nc.dram_tensor("wq_s", [E, HL * A], f32, kind="ExternalInput").ap(),
        "wk_s": nc.dram_tensor("wk_s", [E, HL * A], f32, kind="ExternalInput").ap(),
        "wv_s": nc.dram_tensor("wv_s", [E, HL * A], f32, kind="ExternalInput").ap(),
        "bv_s": nc.dram_tensor("bv_s", [1, HL * A], f32, kind="ExternalInput").ap(),
        "bq_s": nc.dram_tensor("bq_s", [P, 2], f32, kind="ExternalInput").ap(),
        "bk_s": nc.dram_tensor("bk_s", [P, 2], f32, kind="ExternalInput").ap(),
        "wo_s": nc.dram_tensor("wo_s", [A, HL, A], f32, kind="ExternalInput").ap(),
        "dmask": nc.dram_tensor("dmask", [4, P, 512], f32, kind="ExternalInput").ap(),
        "out": nc.dram_tensor("out", [S, A], f32, kind="ExternalOutput").ap(),
    }
    with tile.TileContext(nc) as tc:
        _emit(tc, aps)
    nc.compile()
    _compiled = nc
    return nc


def _install_ntff_shim():
    """run_bass_kernel_spmd's trace path imports antenv.axon_hooks, which
    this container lacks; trn_agent_boot ships the ctypes equivalent."""
    import sys
    import types
    try:
        import antenv.axon_hooks  # noqa: F401
        return
    except ImportError:
        pass
    try:
        from trn_agent_boot.trn_boot import _ntff_profile_via_ctypes
        hook = _ntff_profile_via_ctypes('/opt/axon/libaxon_pjrt.so')
    except Exception:
        hook = None
    mod = types.ModuleType('antenv.axon_hooks')
    mod.get_axon_ntff_profile_hook = lambda: hook
    mod.set_axon_ntff_profile_hook = lambda h: None
    pkg = types.ModuleType('antenv')
    pkg.axon_hooks = mod
    sys.modules.setdefault('antenv', pkg)
    sys.modules['antenv.axon_hooks'] = mod


def kernel(x, mask, wq, bq, wk, bk, wv, bv, wo, bo, _trace=False):
    x, wq, bq, wk, bk, wv, bv, wo, bo = (
        np.asarray(a) for a in (x, wq, bq, wk, bk, wv, bv, wo, bo))
    from concourse.bass_utils import run_bass_kernel_spmd

    nc = _build()
    in_maps = _shard_inputs(x, wq, bq, wk, bk, wv, bv, wo)
    kw = {}
    if _trace:
        _install_ntff_shim()
        kw = dict(trace=True, trace_cores=list(range(N_CORES)))
    res = run_bass_kernel_spmd(nc, in_maps, list(range(N_CORES)), **kw)
    parts = [res.results[c]["out"] for c in range(N_CORES)]
    out = np.stack([parts[2 * b] + parts[2 * b + 1] for b in range(B)])
    out = out + np.asarray(bo, dtype=np.float32)[None, None, :]
    if _trace:
        kernel.last_exec_time_ns = res.exec_time_ns
        kernel.last_mean_exec_time_ns = res.mean_exec_time_ns
        kernel.last_results = res
    return out.astype(np.float32)


# revision 18
# speedup vs baseline: 1.0877x; 1.0855x over previous
"""Multi-head attention (B=4, S=1024, E=512, A=64, H=8) on 8 Trainium2 cores.

Sharding: core c -> (batch b = c//2, head-group g = c%2 covering heads
4g..4g+3). Each core computes its 4 heads end-to-end plus a *partial*
output projection (out = sum_h heads_h @ wo_h decomposes over heads), so
the host just sums core pairs and adds bo. No collectives.

Device layout (per core) keeps everything transposed so no on-device
transposes are needed:
  xt   [E=512, S=1024]  (host passes x^T)
  qT,kT [A=64, S] per head, pair-stacked into [128, 2, 1024] tiles
  v    [S, A] natural, with a ones column appended -> AV matmul row 64
       accumulates the softmax denominator for free
  scores^T tiles [t_chunk=128, s_block<=512]: causal masking skips
       upper-triangle tiles and shortens diagonal ones; only a 128x128
       sub-block per diagonal tile needs an elementwise mask multiply.
"""

import os
import numpy as np

B, S, E, A, H = 4, 1024, 512, 64, 8
HL = 4  # heads per core
N_CORES = 8
P = 128
# fp32r stage mask: bit0 qk/v projections, bit1 scores, bit2 AV, bit3 o-proj
FP32R_MASK = int(os.environ.get("KERNEL_FP32R_MASK", "15"))

_compiled = None


def _f32(x):
    return np.ascontiguousarray(x, dtype=np.float32)


def _build_masks():
    # dmask[d, tp, sf] = 1.0 if (128*d + tp) <= sf else 0 ; d = t_chunk - 4*s_block
    d = np.arange(4)[:, None, None]
    tp = np.arange(P)[None, :, None]
    sf = np.arange(512)[None, None, :]
    return ((P * d + tp) <= sf).astype(np.float32)


def _shard_inputs(x, wq, bq, wk, bk, wv, bv, wo):
    """Build the 8 per-core input dicts."""
    dmask = _build_masks()
    in_maps = []
    for c in range(N_CORES):
        b, g = divmod(c, 2)
        hs = slice(4 * g, 4 * g + 4)
        wq_l, wk_l, wv_l = wq[hs], wk[hs], wv[hs]          # [4, E, A]
        m = {
            "xt": _f32(x[b].T),                             # [E, S]
            "wq_s": _f32(np.transpose(wq_l, (1, 0, 2)).reshape(E, HL * A)),
            "wk_s": _f32(np.transpose(wk_l, (1, 0, 2)).reshape(E, HL * A)),
            "wv_s": _f32(np.transpose(wv_l, (1, 0, 2)).reshape(E, HL * A)),
            "bv_s": _f32(bv[hs].reshape(1, HL * A)),
            "bq_s": _f32(bq[hs].reshape(2, 2 * A).T),       # [128, 2] pair-stacked
            "bk_s": _f32(bk[hs].reshape(2, 2 * A).T),
            "wo_s": _f32(wo[g * HL * A:(g + 1) * HL * A]
                         .reshape(HL, A, A).transpose(1, 0, 2)),  # [64, 4, 64]
            "dmask": dmask,
            "onec": np.ones((1, S), np.float32),
            "vvones": np.ones((P, HL, 8, 1), np.float32),
        }
        in_maps.append(m)
    return in_maps


def _emit(tc, aps):
    from contextlib import ExitStack
    import concourse.bass as bass
    import concourse.mybir as mybir
    from concourse.bass import ts

    nc = tc.nc
    f32 = mybir.dt.float32
    f32r = mybir.dt.float32r
    dt_proj = f32r if FP32R_MASK & 1 else f32    # xt/wq/wk/wv/bv/ones (qk+v matmuls)
    dt_sc = f32r if FP32R_MASK & 2 else f32      # qt/kt (scores matmuls)
    dt_av = f32r if FP32R_MASK & 4 else f32      # vv/p (AV matmuls)
    dt_o = f32r if FP32R_MASK & 8 else f32       # hn/wo (o-proj matmuls)
    Identity = mybir.ActivationFunctionType.Identity
    Exp = mybir.ActivationFunctionType.Exp

    def mm(out, lhsT, rhs, **kw):
        return nc.tensor.matmul(out, lhsT=lhsT, rhs=rhs, **kw)

    def md(dram_ap, dt):
        return dram_ap.bitcast(dt) if dt is f32r else dram_ap

    xt, wq_s, wk_s, wv_s, bv_s, bq_s, bk_s, wo_s, dmask, out = (
        aps["xt"], aps["wq_s"], aps["wk_s"], aps["wv_s"], aps["bv_s"],
        aps["bq_s"], aps["bk_s"], aps["wo_s"], aps["dmask"], aps["out"],
    )
    onec, vvones = aps["onec"], aps["vvones"]

    ctx = ExitStack()
    const = ctx.enter_context(tc.tile_pool(name="const", bufs=1))
    ppool = ctx.enter_context(tc.tile_pool(name="p", bufs=10))
    rpool = ctx.enter_context(tc.tile_pool(name="r", bufs=2))
    dpool = ctx.enter_context(tc.tile_pool(name="dram", bufs=2, space="DRAM"))
    mm_ps = ctx.enter_context(tc.tile_pool(name="mmps", bufs=2, space="PSUM"))
    sc_ps = ctx.enter_context(tc.tile_pool(name="scps", bufs=3, space="PSUM"))
    av_ps = ctx.enter_context(tc.tile_pool(name="avps", bufs=1, space="PSUM"))
    o_ps = ctx.enter_context(tc.tile_pool(name="ops", bufs=1, space="PSUM"))

    # ---- constant loads ----
    xt_sb = const.tile([P, 4, S], dt_proj, tag="xt")
    nc.sync.dma_start(xt_sb[:], md(xt.rearrange("(k p) s -> p k s", p=P), dt_proj))
    wq_sb = const.tile([P, 4, 256], dt_proj, tag="wq")
    nc.sync.dma_start(wq_sb[:], md(wq_s.rearrange("(k p) n -> p k n", p=P), dt_proj))
    wk_sb = const.tile([P, 4, 256], dt_proj, tag="wk")
    nc.sync.dma_start(wk_sb[:], md(wk_s.rearrange("(k p) n -> p k n", p=P), dt_proj))
    wv_sb = const.tile([P, 4, 256], dt_proj, tag="wv")
    nc.sync.dma_start(wv_sb[:], md(wv_s.rearrange("(k p) n -> p k n", p=P), dt_proj))
    bv_sb = const.tile([1, 256], dt_proj, tag="bv")
    nc.sync.dma_start(bv_sb[:], md(bv_s[:], dt_proj))
    bq_sb = const.tile([P, 2], f32, tag="bq")
    nc.sync.dma_start(bq_sb[:], bq_s[:])
    bk_sb = const.tile([P, 2], f32, tag="bk")
    nc.sync.dma_start(bk_sb[:], bk_s[:])
    wo_sb = const.tile([64, 4, 64], dt_o, tag="wo")
    nc.sync.dma_start(wo_sb[:], md(wo_s[:], dt_o))
    mk_sb = const.tile([P, 4, 512], f32, tag="mk")
    nc.sync.dma_start(mk_sb[:], dmask.rearrange("d p s -> p d s"))

    ones_sb = const.tile([1, S], dt_proj, tag="ones")
    nc.sync.dma_start(ones_sb[:], md(onec[:], dt_proj))

    qt_sb = const.tile([P, 2, S], dt_sc, tag="qt")   # pair-stacked q^T
    kt_sb = const.tile([P, 2, S], dt_sc, tag="kt")
    vv_sb = const.tile([P, HL, 8, A + 1], dt_av, tag="vv")  # v|ones per head/t-chunk
    nc.sync.dma_start(vv_sb[:, :, :, A:A + 1], md(vvones[:], dt_av))
    hn_sb = [const.tile([A, S], dt_o, tag=f"hn{h}", name=f"hn{h}") for h in range(HL)]
    osb = const.tile([P, 8, A], f32, tag="osb")

    # ---- V projection: v[s, a] for all 4 heads at once (N=256) ----
    for t in range(8):
        pv = mm_ps.tile([P, 512], f32, tag="mm")
        for k in range(4):
            mm(pv[:, :256], xt_sb[:, k, ts(t, P)],
                             wv_sb[:, k, :], start=(k == 0), stop=False)
        mm(pv[:, :256], ones_sb[:, ts(t, P)],
                         bv_sb[:], start=False, stop=True)
        for h in range(HL):
            nc.vector.tensor_copy(vv_sb[:, h, t, 0:A], pv[:, A * h:A * h + A])

    # ---- Q/K projections: q^T, k^T pair-stacked [128, S] (2 heads x 64) ----
    for wsb, bsb, dst in ((wq_sb, bq_sb, qt_sb), (wk_sb, bk_sb, kt_sb)):
        for p in range(2):
            for blk in range(2):
                pq = mm_ps.tile([P, 512], f32, tag="mm")
                for k in range(4):
                    mm(pq[:, :], wsb[:, k, 128 * p:128 * p + 128],
                                     xt_sb[:, k, ts(blk, 512)],
                                     start=(k == 0), stop=(k == 3))
                nc.scalar.activation(dst[:, p, ts(blk, 512)], pq[:, :], Identity,
                                     bias=bsb[:, p:p + 1], scale=1.0)

    # ---- attention per head pair; scores^T layout [t, s] ----
    for p in range(2):
        for j in range(2):  # s_block
            n_i = 4 * j + 4
            pts = {}
            for i in range(n_i):  # t_chunk
                d = i - 4 * j
                col0 = 128 * d if d > 0 else 0
                for loc in range(2):
                    pr = 64 * loc
                    sc = sc_ps.tile([P, 512], f32, tag="sc")
                    mm(sc[:, col0:], kt_sb[pr:pr + 64, p, ts(i, P)],
                                     qt_sb[pr:pr + 64, p, 512 * j + col0:512 * (j + 1)],
                                     start=True, stop=True)
                    pt = ppool.tile([P, 512], dt_av, tag="p")
                    nc.scalar.activation(pt[:, col0:], sc[:, col0:], Exp, scale=0.125)
                    if d >= 0:
                        nc.vector.tensor_mul(pt[:, col0:col0 + 128], pt[:, col0:col0 + 128],
                                             mk_sb[:, d, col0:col0 + 128])
                    pts[(i, loc)] = pt
            for loc in range(2):
                h = 2 * p + loc
                av = av_ps.tile([A + 1, 512], f32, tag="av")
                for i in range(n_i):
                    d = i - 4 * j
                    col0 = 128 * d if d > 0 else 0
                    mm(av[:, col0:], vv_sb[:, h, i, :],
                                     pts[(i, loc)][:, col0:],
                                     start=(i == 0), stop=(i == n_i - 1))
                # softmax denominators: reciprocal of av row 64, then
                # partition-broadcast. A [1,512] reciprocal runs on a single
                # DVE lane (8 cyc/elem, ~3.4us), so bounce through DRAM to
                # reshape the row across 64 lanes first; DRAM sources may
                # carry stride-0 partition dims for the final broadcast.
                rt = rpool.tile([P, 512], f32, tag="r")
                rs = rpool.tile([A, 8], f32, tag="rs")
                nc.scalar.copy(rt[64:65, :], av[64:65, :])
                dsum = dpool.tile([A, 8], f32, tag="dsum")
                drec = dpool.tile([1, 512], f32, tag="drec")
                nc.sync.dma_start(dsum[:], rt[64:65, :])
                nc.sync.dma_start(rs[:], dsum[:])
                nc.vector.reciprocal(rs[:], rs[:])
                nc.sync.dma_start(drec[:], rs[:])
                nc.sync.dma_start(rt[0:64, :], drec[0:1, :].to_broadcast((64, 512)))
                nc.vector.tensor_mul(hn_sb[h][:, ts(j, 512)], av[0:A, :], rt[0:64, :])

    # ---- partial output projection: out[s, :] = sum_h heads_h @ wo_h ----
    for t in range(8):
        po = o_ps.tile([P, A], f32, tag="o")
        for h in range(HL):
            mm(po[:], hn_sb[h][:, ts(t, P)], wo_sb[:, h, :],
                             start=(h == 0), stop=(h == HL - 1))
        nc.vector.tensor_copy(osb[:, t, :], po[:])
    nc.sync.dma_start(out.rearrange("(c p) n -> p c n", p=P), osb[:])
    ctx.close()


def _build():
    global _compiled
    if _compiled is not None:
        return _compiled
    import concourse.bacc as bacc
    import concourse.mybir as mybir
    import concourse.tile as tile

    nc = bacc.Bacc("TRN2", target_bir_lowering=False, debug=False,
                   num_devices=N_CORES)
    f32 = mybir.dt.float32
    aps = {
        "xt": nc.dram_tensor("xt", [E, S], f32, kind="ExternalInput").ap(),
        "wq_s": nc.dram_tensor("wq_s", [E, HL * A], f32, kind="ExternalInput").ap(),
        "wk_s": nc.dram_tensor("wk_s", [E, HL * A], f32, kind="ExternalInput").ap(),
        "wv_s": nc.dram_tensor("wv_s", [E, HL * A], f32, kind="ExternalInput").ap(),
        "bv_s": nc.dram_tensor("bv_s", [1, HL * A], f32, kind="ExternalInput").ap(),
        "bq_s": nc.dram_tensor("bq_s", [P, 2], f32, kind="ExternalInput").ap(),
        "bk_s": nc.dram_tensor("bk_s", [P, 2], f32, kind="ExternalInput").ap(),
        "wo_s": nc.dram_tensor("wo_s", [A, HL, A], f32, kind="ExternalInput").ap(),
        "dmask": nc.dram_tensor("dmask", [4, P, 512], f32, kind="ExternalInput").ap(),
        "onec": nc.dram_tensor("onec", [1, S], f32, kind="ExternalInput").ap(),
        "vvones": nc.dram_tensor("vvones", [P, HL, 8, 1], f32, kind="ExternalInput").ap(),
        "out": nc.dram_tensor("out", [S, A], f32, kind="ExternalOutput").ap(),
    }
    with tile.TileContext(nc) as tc:
        _emit(tc, aps)
    nc.compile()
    _compiled = nc
    return nc


def _install_ntff_shim():
    """run_bass_kernel_spmd's trace path imports antenv.axon_hooks, which
    this container lacks; trn_agent_boot ships the ctypes equivalent."""
    import sys
    import types
    try:
        import antenv.axon_hooks  # noqa: F401
        return
    except ImportError:
        pass
    try:
        from trn_agent_boot.trn_boot import _ntff_profile_via_ctypes
        hook = _ntff_profile_via_ctypes('/opt/axon/libaxon_pjrt.so')
    except Exception:
        hook = None
    mod = types.ModuleType('antenv.axon_hooks')
    mod.get_axon_ntff_profile_hook = lambda: hook
    mod.set_axon_ntff_profile_hook = lambda h: None
    pkg = types.ModuleType('antenv')
    pkg.axon_hooks = mod
    sys.modules.setdefault('antenv', pkg)
    sys.modules['antenv.axon_hooks'] = mod


def kernel(x, mask, wq, bq, wk, bk, wv, bv, wo, bo, _trace=False, _trace_cores=None):
    x, wq, bq, wk, bk, wv, bv, wo, bo = (
        np.asarray(a) for a in (x, wq, bq, wk, bk, wv, bv, wo, bo))
    from concourse.bass_utils import run_bass_kernel_spmd

    nc = _build()
    in_maps = _shard_inputs(x, wq, bq, wk, bk, wv, bv, wo)
    kw = {}
    if _trace:
        _install_ntff_shim()
        kw = dict(trace=True,
                  trace_cores=list(range(N_CORES)) if _trace_cores is None else _trace_cores)
    res = run_bass_kernel_spmd(nc, in_maps, list(range(N_CORES)), **kw)
    parts = [res.results[c]["out"] for c in range(N_CORES)]
    out = np.stack([parts[2 * b] + parts[2 * b + 1] for b in range(B)])
    out = out + np.asarray(bo, dtype=np.float32)[None, None, :]
    if _trace:
        kernel.last_exec_time_ns = res.exec_time_ns
        kernel.last_mean_exec_time_ns = res.mean_exec_time_ns
        kernel.last_results = res
    return out.astype(np.float32)


# revision 20
# speedup vs baseline: 1.2295x; 1.1304x over previous
"""Multi-head attention (B=4, S=1024, E=512, A=64, H=8) on 8 Trainium2 cores.

Sharding: core c -> (batch b = c//2, head-group g = c%2 covering heads
4g..4g+3). Each core computes its 4 heads end-to-end plus a *partial*
output projection (out = sum_h heads_h @ wo_h decomposes over heads), so
the host just sums core pairs and adds bo. No collectives.

Device layout (per core) keeps everything transposed so no on-device
transposes are needed:
  xt   [E=512, S=1024]  (host passes x^T)
  qT,kT [A=64, S] per head, pair-stacked into [128, 2, 1024] tiles
  v    [S, A] natural, with a ones column appended -> AV matmul row 64
       accumulates the softmax denominator for free
  scores^T tiles [t_chunk=128, s_block<=512]: causal masking skips
       upper-triangle tiles and shortens diagonal ones; only a 128x128
       sub-block per diagonal tile needs an elementwise mask multiply.
"""

import os
import numpy as np

B, S, E, A, H = 4, 1024, 512, 64, 8
HL = 4  # heads per core
N_CORES = 8
P = 128
# fp32r stage mask: bit0 qk/v projections, bit1 scores, bit2 AV, bit3 o-proj
FP32R_MASK = int(os.environ.get("KERNEL_FP32R_MASK", "15"))

_compiled = None


def _f32(x):
    return np.ascontiguousarray(x, dtype=np.float32)


def _build_masks():
    # dmask[d, tp, sf] = 1.0 if (128*d + tp) <= sf else 0 ; d = t_chunk - 4*s_block
    d = np.arange(4)[:, None, None]
    tp = np.arange(P)[None, :, None]
    sf = np.arange(512)[None, None, :]
    return ((P * d + tp) <= sf).astype(np.float32)


def _shard_inputs(x, wq, bq, wk, bk, wv, bv, wo):
    """Build the 8 per-core input dicts."""
    dmask = _build_masks()
    in_maps = []
    for c in range(N_CORES):
        b, g = divmod(c, 2)
        hs = slice(4 * g, 4 * g + 4)
        wq_l, wk_l, wv_l = wq[hs], wk[hs], wv[hs]          # [4, E, A]
        m = {
            "xt": _f32(x[b].T),                             # [E, S]
            "wq_s": _f32(np.transpose(wq_l, (1, 0, 2)).reshape(E, HL * A)),
            "wk_s": _f32(np.transpose(wk_l, (1, 0, 2)).reshape(E, HL * A)),
            "wv_s": _f32(np.transpose(wv_l, (1, 0, 2)).reshape(E, HL * A)),
            "bv_s": _f32(bv[hs].reshape(1, HL * A)),
            "bq_s": _f32(bq[hs].reshape(2, 2 * A).T),       # [128, 2] pair-stacked
            "bk_s": _f32(bk[hs].reshape(2, 2 * A).T),
            "wo_s": _f32(wo[g * HL * A:(g + 1) * HL * A]
                         .reshape(HL, A, A).transpose(1, 0, 2)),  # [64, 4, 64]
            "dmask": dmask,
            "onec": np.ones((1, S), np.float32),
            "vvtail": np.concatenate([np.ones((P, HL, 8, 1), np.float32),
                                      np.zeros((P, HL, 8, A - 1), np.float32)], axis=3),
        }
        in_maps.append(m)
    return in_maps


def _emit(tc, aps):
    from contextlib import ExitStack
    import concourse.bass as bass
    import concourse.mybir as mybir
    from concourse.bass import ts

    nc = tc.nc
    f32 = mybir.dt.float32
    f32r = mybir.dt.float32r
    dt_proj = f32r if FP32R_MASK & 1 else f32    # xt/wq/wk/wv/bv/ones (qk+v matmuls)
    dt_sc = f32r if FP32R_MASK & 2 else f32      # qt/kt (scores matmuls)
    dt_av = f32r if FP32R_MASK & 4 else f32      # vv/p (AV matmuls)
    dt_o = f32r if FP32R_MASK & 8 else f32       # hn/wo (o-proj matmuls)
    Identity = mybir.ActivationFunctionType.Identity
    Exp = mybir.ActivationFunctionType.Exp

    def mm(out, lhsT, rhs, **kw):
        return nc.tensor.matmul(out, lhsT=lhsT, rhs=rhs, **kw)

    def md(dram_ap, dt):
        return dram_ap.bitcast(dt) if dt is f32r else dram_ap

    xt, wq_s, wk_s, wv_s, bv_s, bq_s, bk_s, wo_s, dmask, out = (
        aps["xt"], aps["wq_s"], aps["wk_s"], aps["wv_s"], aps["bv_s"],
        aps["bq_s"], aps["bk_s"], aps["wo_s"], aps["dmask"], aps["out"],
    )
    onec, vvtail = aps["onec"], aps["vvtail"]

    ctx = ExitStack()
    const = ctx.enter_context(tc.tile_pool(name="const", bufs=1))
    ppool = ctx.enter_context(tc.tile_pool(name="p", bufs=10))
    rpool = ctx.enter_context(tc.tile_pool(name="r", bufs=2))
    dpool = ctx.enter_context(tc.tile_pool(name="dram", bufs=2, space="DRAM"))
    mm_ps = ctx.enter_context(tc.tile_pool(name="mmps", bufs=2, space="PSUM"))
    sc_ps = ctx.enter_context(tc.tile_pool(name="scps", bufs=3, space="PSUM"))
    av_ps = ctx.enter_context(tc.tile_pool(name="avps", bufs=1, space="PSUM"))
    o_ps = ctx.enter_context(tc.tile_pool(name="ops", bufs=1, space="PSUM"))

    # ---- constant loads ----
    xt_sb = const.tile([P, 4, S], dt_proj, tag="xt")
    nc.sync.dma_start(xt_sb[:], md(xt.rearrange("(k p) s -> p k s", p=P), dt_proj))
    wq_sb = const.tile([P, 4, 256], dt_proj, tag="wq")
    nc.sync.dma_start(wq_sb[:], md(wq_s.rearrange("(k p) n -> p k n", p=P), dt_proj))
    wk_sb = const.tile([P, 4, 256], dt_proj, tag="wk")
    nc.sync.dma_start(wk_sb[:], md(wk_s.rearrange("(k p) n -> p k n", p=P), dt_proj))
    wv_sb = const.tile([P, 4, 256], dt_proj, tag="wv")
    nc.sync.dma_start(wv_sb[:], md(wv_s.rearrange("(k p) n -> p k n", p=P), dt_proj))
    bv_sb = const.tile([1, 256], dt_proj, tag="bv")
    nc.sync.dma_start(bv_sb[:], md(bv_s[:], dt_proj))
    bq_sb = const.tile([P, 2], f32, tag="bq")
    nc.sync.dma_start(bq_sb[:], bq_s[:])
    bk_sb = const.tile([P, 2], f32, tag="bk")
    nc.sync.dma_start(bk_sb[:], bk_s[:])
    wo_sb = const.tile([64, 4, 64], dt_o, tag="wo")
    nc.sync.dma_start(wo_sb[:], md(wo_s[:], dt_o))
    mk_sb = const.tile([P, 4, 512], f32, tag="mk")
    nc.sync.dma_start(mk_sb[:], dmask.rearrange("d p s -> p d s"))

    ones_sb = const.tile([1, S], dt_proj, tag="ones")
    nc.sync.dma_start(ones_sb[:], md(onec[:], dt_proj))

    qt_sb = const.tile([P, 2, S], dt_sc, tag="qt")   # pair-stacked q^T
    kt_sb = const.tile([P, 2, S], dt_sc, tag="kt")
    # v | ones-column | zero padding, per head/t-chunk; padded to M=128 because
    # fp32r matmuls reject partial dst partition ranges (65 wedges the exec unit)
    vv_sb = const.tile([P, HL, 8, 2 * A], dt_av, tag="vv")
    nc.sync.dma_start(vv_sb[:, :, :, A:2 * A], md(vvtail[:], dt_av))
    hn_sb = [const.tile([A, S], dt_o, tag=f"hn{h}", name=f"hn{h}") for h in range(HL)]
    osb = const.tile([P, 8, A], f32, tag="osb")

    # ---- V projection: v[s, a] for all 4 heads at once (N=256) ----
    for t in range(8):
        pv = mm_ps.tile([P, 512], f32, tag="mm")
        for k in range(4):
            mm(pv[:, :256], xt_sb[:, k, ts(t, P)],
                             wv_sb[:, k, :], start=(k == 0), stop=False)
        mm(pv[:, :256], ones_sb[:, ts(t, P)],
                         bv_sb[:], start=False, stop=True)
        for h in range(HL):
            nc.vector.tensor_copy(vv_sb[:, h, t, 0:A], pv[:, A * h:A * h + A])

    # ---- Q/K projections: q^T, k^T pair-stacked [128, S] (2 heads x 64) ----
    for wsb, bsb, dst in ((wq_sb, bq_sb, qt_sb), (wk_sb, bk_sb, kt_sb)):
        for p in range(2):
            for blk in range(2):
                pq = mm_ps.tile([P, 512], f32, tag="mm")
                for k in range(4):
                    mm(pq[:, :], wsb[:, k, 128 * p:128 * p + 128],
                                     xt_sb[:, k, ts(blk, 512)],
                                     start=(k == 0), stop=(k == 3))
                nc.scalar.activation(dst[:, p, ts(blk, 512)], pq[:, :], Identity,
                                     bias=bsb[:, p:p + 1], scale=1.0)

    # ---- attention per head pair; scores^T layout [t, s] ----
    for p in range(2):
        for j in range(2):  # s_block
            n_i = 4 * j + 4
            pts = {}
            for i in range(n_i):  # t_chunk
                d = i - 4 * j
                col0 = 128 * d if d > 0 else 0
                for loc in range(2):
                    pr = 64 * loc
                    sc = sc_ps.tile([P, 512], f32, tag="sc")
                    mm(sc[:, col0:], kt_sb[pr:pr + 64, p, ts(i, P)],
                                     qt_sb[pr:pr + 64, p, 512 * j + col0:512 * (j + 1)],
                                     start=True, stop=True)
                    pt = ppool.tile([P, 512], dt_av, tag="p")
                    nc.scalar.activation(pt[:, col0:], sc[:, col0:], Exp, scale=0.125)
                    if d >= 0:
                        nc.vector.tensor_mul(pt[:, col0:col0 + 128], pt[:, col0:col0 + 128],
                                             mk_sb[:, d, col0:col0 + 128])
                    pts[(i, loc)] = pt
            for loc in range(2):
                h = 2 * p + loc
                av = av_ps.tile([P, 512], f32, tag="av")
                for i in range(n_i):
                    d = i - 4 * j
                    col0 = 128 * d if d > 0 else 0
                    mm(av[:, col0:], vv_sb[:, h, i, :],
                                     pts[(i, loc)][:, col0:],
                                     start=(i == 0), stop=(i == n_i - 1))
                # softmax denominators: reciprocal of av row 64, then
                # partition-broadcast. A [1,512] reciprocal runs on a single
                # DVE lane (8 cyc/elem, ~3.4us), so bounce through DRAM to
                # reshape the row across 64 lanes first; DRAM sources may
                # carry stride-0 partition dims for the final broadcast.
                rt = rpool.tile([P, 512], f32, tag="r")
                rs = rpool.tile([A, 8], f32, tag="rs")
                nc.scalar.copy(rt[64:65, :], av[64:65, :])
                dsum = dpool.tile([A, 8], f32, tag="dsum")
                drec = dpool.tile([1, 512], f32, tag="drec")
                nc.sync.dma_start(dsum[:], rt[64:65, :])
                nc.sync.dma_start(rs[:], dsum[:])
                nc.vector.reciprocal(rs[:], rs[:])
                nc.sync.dma_start(drec[:], rs[:])
                nc.sync.dma_start(rt[0:64, :], drec[0:1, :].to_broadcast((64, 512)))
                nc.vector.tensor_mul(hn_sb[h][:, ts(j, 512)], av[0:A, :], rt[0:64, :])

    # ---- partial output projection: out[s, :] = sum_h heads_h @ wo_h ----
    for t in range(8):
        po = o_ps.tile([P, A], f32, tag="o")
        for h in range(HL):
            mm(po[:], hn_sb[h][:, ts(t, P)], wo_sb[:, h, :],
                             start=(h == 0), stop=(h == HL - 1))
        nc.vector.tensor_copy(osb[:, t, :], po[:])
    nc.sync.dma_start(out.rearrange("(c p) n -> p c n", p=P), osb[:])
    ctx.close()


def _build():
    global _compiled
    if _compiled is not None:
        return _compiled
    import concourse.bacc as bacc
    import concourse.mybir as mybir
    import concourse.tile as tile

    nc = bacc.Bacc("TRN2", target_bir_lowering=False, debug=False,
                   num_devices=N_CORES)
    f32 = mybir.dt.float32
    aps = {
        "xt": nc.dram_tensor("xt", [E, S], f32, kind="ExternalInput").ap(),
        "wq_s": nc.dram_tensor("wq_s", [E, HL * A], f32, kind="ExternalInput").ap(),
        "wk_s": nc.dram_tensor("wk_s", [E, HL * A], f32, kind="ExternalInput").ap(),
        "wv_s": nc.dram_tensor("wv_s", [E, HL * A], f32, kind="ExternalInput").ap(),
        "bv_s": nc.dram_tensor("bv_s", [1, HL * A], f32, kind="ExternalInput").ap(),
        "bq_s": nc.dram_tensor("bq_s", [P, 2], f32, kind="ExternalInput").ap(),
        "bk_s": nc.dram_tensor("bk_s", [P, 2], f32, kind="ExternalInput").ap(),
        "wo_s": nc.dram_tensor("wo_s", [A, HL, A], f32, kind="ExternalInput").ap(),
        "dmask": nc.dram_tensor("dmask", [4, P, 512], f32, kind="ExternalInput").ap(),
        "onec": nc.dram_tensor("onec", [1, S], f32, kind="ExternalInput").ap(),
        "vvtail": nc.dram_tensor("vvtail", [P, HL, 8, A], f32, kind="ExternalInput").ap(),
        "out": nc.dram_tensor("out", [S, A], f32, kind="ExternalOutput").ap(),
    }
    with tile.TileContext(nc) as tc:
        _emit(tc, aps)
    nc.compile()
    _compiled = nc
    return nc


def _install_ntff_shim():
    """run_bass_kernel_spmd's trace path imports antenv.axon_hooks, which
    this container lacks; trn_agent_boot ships the ctypes equivalent."""
    import sys
    import types
    try:
        import antenv.axon_hooks  # noqa: F401
        return
    except ImportError:
        pass
    try:
        from trn_agent_boot.trn_boot import _ntff_profile_via_ctypes
        hook = _ntff_profile_via_ctypes('/opt/axon/libaxon_pjrt.so')
    except Exception:
        hook = None
    mod = types.ModuleType('antenv.axon_hooks')
    mod.get_axon_ntff_profile_hook = lambda: hook
    mod.set_axon_ntff_profile_hook = lambda h: None
    pkg = types.ModuleType('antenv')
    pkg.axon_hooks = mod
    sys.modules.setdefault('antenv', pkg)
    sys.modules['antenv.axon_hooks'] = mod


def kernel(x, mask, wq, bq, wk, bk, wv, bv, wo, bo, _trace=False, _trace_cores=None):
    x, wq, bq, wk, bk, wv, bv, wo, bo = (
        np.asarray(a) for a in (x, wq, bq, wk, bk, wv, bv, wo, bo))
    from concourse.bass_utils import run_bass_kernel_spmd

    nc = _build()
    in_maps = _shard_inputs(x, wq, bq, wk, bk, wv, bv, wo)
    kw = {}
    if _trace:
        _install_ntff_shim()
        kw = dict(trace=True,
                  trace_cores=list(range(N_CORES)) if _trace_cores is None else _trace_cores)
    res = run_bass_kernel_spmd(nc, in_maps, list(range(N_CORES)), **kw)
    parts = [res.results[c]["out"] for c in range(N_CORES)]
    out = np.stack([parts[2 * b] + parts[2 * b + 1] for b in range(B)])
    out = out + np.asarray(bo, dtype=np.float32)[None, None, :]
    if _trace:
        kernel.last_exec_time_ns = res.exec_time_ns
        kernel.last_mean_exec_time_ns = res.mean_exec_time_ns
        kernel.last_results = res
    return out.astype(np.float32)
